# revision 20
# baseline (speedup 1.0000x reference)
"""Single-head causal attention (B=4, S=2048, D=1024, dk=128) on 8 TRN2 cores.

Sharding: core c -> batch b=c//2, half h=c%2. Query rows are split at
256-row granularity into blocks; core h owns blocks (0,3,4,7) or
(1,2,5,6) of the 8 blocks, ordered by causal reach so the two cores run
one identical program whose per-slot key-tile counts are the positional
max (4,8,12,16) of both cores' needs — 40 key-tiles vs the 48 a
512-block split needs.

Activations are passed pre-transposed to [d_model, s] and pre-cast to
fp8 E3M4 (1 byte/elem, 4 mantissa bits); weights stay bf16. The PE
multiplies bf16 stationary x fp8e3 moving directly (verified exact vs
emulation), so fp8 halves HBM traffic without an upcast pass or extra
matmul error beyond the e3m4 quantization itself (~1.1e-2 rel, vs the
2e-2 gate). Scores are computed transposed ([key, query]) so the P@V
matmul consumes P tiles as the stationary operand and V in natural
[s, dk] layout; a ones-column appended to V makes the same matmul
accumulate the softmax denominators. The causal mask is applied as a
multiplicative bf16 mask on P built on-chip from a per-core [128, 16]
shift table: mask tile j covers query cols [soff(j), soff(j)+256) and
is all-ones / diagonal / all-zero as needed, so one program serves both
halves.
"""

import math

import numpy as np
import ml_dtypes

import concourse.bacc as bacc
import concourse.tile as tile
import concourse.mybir as mybir
from concourse import bass_utils
from concourse.masks import make_identity
from concourse.tile_rust import add_dep_helper

F32 = mybir.dt.float32
BF16 = mybir.dt.bfloat16
E3 = mybir.dt.float8e3

B, S, DM, DK = 4, 2048, 1024, 128
NCORES = 8
HALF = S // 2          # query rows per core
NCH = DM // 128        # d_model chunks
NJT = S // 128         # key tiles (16)
QB = 256               # query block rows
NSLOT = HALF // QB     # 4 query-block slots per core
NJ = (4, 8, 12, 16)    # key tiles computed for slot j//4
SOFF = (0, 256, 512, 768)  # first query col that still needs key tile j
VW = DK + 1            # v tiles carry a ones-column for the denominator
SCALE = 1.0 / math.sqrt(DK)
# per-core query blocks, ordered so both cores' causal needs fit (4,8,12,16)
BLOCKS = ((0, 3, 4, 7), (1, 2, 5, 6))
WARMUP_MMS = 24
FILLER_MMS = 2

_CACHE = {}


def _build():
    if "nc" in _CACHE:
        return _CACHE["nc"]
    nc = bacc.Bacc("TRN2", target_bir_lowering=False, debug=False, num_devices=NCORES)

    # packed host layouts: one [128, NCH, 512] contiguous block per 512
    # activation columns, so each projection block gates only on its own slice
    qx_in = nc.dram_tensor("qx", [2, 128, NCH, 512], E3, kind="ExternalInput").ap()
    kx_in = nc.dram_tensor("kx", [4, 128, NCH, 512], E3, kind="ExternalInput").ap()
    vx_in = nc.dram_tensor("vx", [4, 128, NCH, 512], E3, kind="ExternalInput").ap()
    wqT_in = nc.dram_tensor("wqT", [128, NCH, DK], BF16, kind="ExternalInput").ap()
    wkT_in = nc.dram_tensor("wkT", [128, NCH, DK], BF16, kind="ExternalInput").ap()
    wvT_in = nc.dram_tensor("wvT", [128, NCH, DK], BF16, kind="ExternalInput").ap()
    shifts_in = nc.dram_tensor("shifts", [128, NJT], F32, kind="ExternalInput").ap()
    out = nc.dram_tensor("out", [NSLOT, 128, 2, DK], BF16, kind="ExternalOutput").ap()

    rings = [nc.sync, nc.scalar, nc.gpsimd]

    with tile.TileContext(nc) as tc:
        with tc.tile_pool(name="const", bufs=1) as const:
            ident = const.tile([128, 128], BF16)
            make_identity(nc, ident)

            # ---- loads: per-(chunk, col-block) DMAs (1-2KB contiguous per
            # partition row), round-robined over 3 issue rings in need order.
            wTs = {}
            for nm, w_dram in (("wq", wqT_in), ("wk", wkT_in), ("wv", wvT_in)):
                wTs[nm] = const.tile([128, NCH, DK], BF16, tag=f"wT_{nm}", name=f"wT{nm}")
            shifts = const.tile([128, NJT], F32)
            iota_i = const.tile([128, QB], mybir.dt.int32)
            nc.gpsimd.iota(iota_i, pattern=[[1, QB]], base=0, channel_multiplier=0)
            iota_f = const.tile([128, QB], F32)
            nc.vector.tensor_copy(iota_f, iota_i)

            qxb = [const.tile([128, NCH, 512], E3, tag=f"qx{b}", name=f"qxb{b}") for b in range(2)]
            kxb = [const.tile([128, NCH, 512], E3, tag=f"kx{b}", name=f"kxb{b}") for b in range(4)]
            vxb = [const.tile([128, NCH, 512], E3, tag=f"vx{b}", name=f"vxb{b}") for b in range(4)]
            ri = 0

            def ld(dst, src):
                nonlocal ri
                rings[ri % len(rings)].dma_start(out=dst, in_=src)
                ri += 1

            # scalar (ACT) also runs the exp chain mid-kernel, so its DMA
            # queue is the straggler: give it only the early q slices and the
            # last v slice; stream kx/vx over sync+gpsimd in need order.
            h0, h1 = slice(0, 4), slice(4, 8)

            def ld2(eng, dst, src, cs):
                eng.dma_start(out=dst[:, cs, :], in_=src[:, cs, :])

            nc.scalar.dma_start(out=wTs["wq"], in_=wqT_in)
            nc.gpsimd.dma_start(out=shifts, in_=shifts_in)
            nc.sync.dma_start(out=wTs["wk"], in_=wkT_in)
            for b in range(2):
                ld2(nc.scalar, qxb[b], qx_in[b], h0)
                ld2(nc.gpsimd, qxb[b], qx_in[b], h1)
            for b in range(4):
                ld2(nc.sync, kxb[b], kx_in[b], h0)
                ld2(nc.gpsimd, kxb[b], kx_in[b], h1)
            nc.scalar.dma_start(out=wTs["wv"], in_=wvT_in)
            for b in range(4):
                ld2(nc.sync, vxb[b], vx_in[b], h0)
                ld2(nc.scalar if b == 3 else nc.gpsimd, vxb[b], vx_in[b], h1)

            # ---- causal masks built on-chip: mask[p, j, c] = (c >= shift[p, j])
            masks_sb = const.tile([128, NJT * QB], BF16)
            for j in range(NJT):
                nc.vector.tensor_scalar(
                    masks_sb[:, j * QB:(j + 1) * QB],
                    iota_f,
                    shifts[:, j:j + 1],
                    None,
                    op0=mybir.AluOpType.is_ge,
                )

            # ---- PE warmup + filler: keep the PE p-state ramp going while
            # the first loads land.
            w_warm = const.tile([128, 512], BF16)
            nc.vector.memset(w_warm, 1.0)
            last_filler = None
            with tc.tile_pool(name="psW", bufs=1, space="PSUM") as psW:
                ps_w = psW.tile([128, 512], F32)
                for _ in range(WARMUP_MMS):
                    nc.tensor.matmul(ps_w[:, 0:128], ident, ident, start=True, stop=True)
                for _ in range(FILLER_MMS):
                    last_filler = nc.tensor.matmul(ps_w, ident, w_warm, start=True, stop=True)

            # ---- persistent projected tensors ----
            qT_sb = const.tile([128, HALF], BF16)
            kT_sb = const.tile([128, S], BF16)
            vsb = const.tile([128, NJT, VW], BF16)

            with (
                tc.tile_pool(name="psM", bufs=2, space="PSUM") as psM,
                tc.tile_pool(name="psS", bufs=2, space="PSUM") as psS,
                tc.tile_pool(name="psO", bufs=4, space="PSUM") as psO,
                tc.tile_pool(name="pP", bufs=26) as p_pool,
                tc.tile_pool(name="oo", bufs=4) as o_pool,
            ):

                def project_block(wT, xT, dst, dst0, xoff, w=512):
                    """dst[:, dst0:dst0+w] bf16 = W @ X^T[:, xoff:xoff+w]."""
                    acc = psM.tile([128, 512], F32, tag="ps_misc", name="acc")
                    for c in range(NCH):
                        mm = nc.tensor.matmul(
                            acc[:, 0:w],
                            wT[:, c, :],
                            xT[:, c, xoff:xoff + w],
                            start=(c == 0),
                            stop=(c == NCH - 1),
                        )
                        if c == 0 and last_filler is not None:
                            add_dep_helper(
                                mm.ins, last_filler.ins, sync=False,
                                reason="run filler first",
                            )
                    nc.vector.tensor_copy(dst[:, dst0:dst0 + w], acc[:, 0:w])

                # p pieces: p_tiles[j] = list of (tile, width) covering query
                # cols [soff(j), HALF) in 512-wide pieces
                p_tiles = {j: [] for j in range(NJT)}

                def scores(j):
                    """score tile j over query cols [soff, HALF): [key, query]
                    transposed, exp, causal mask on the first 256 cols."""
                    soff = SOFF[j // 4]
                    kt = kT_sb[:, j * 128:(j + 1) * 128]
                    first = True
                    for q0 in range(soff, HALF, 512):
                        w = min(512, HALF - q0)
                        ps_s = psS.tile([128, 512], F32, tag="score")
                        nc.tensor.matmul(
                            ps_s[:, 0:w], kt, qT_sb[:, q0:q0 + w],
                            start=True, stop=True,
                        )
                        p_t = p_pool.tile([128, 512], BF16, tag="p")
                        nc.scalar.activation(
                            p_t[:, 0:w], ps_s[:, 0:w],
                            mybir.ActivationFunctionType.Exp, scale=SCALE,
                        )
                        if first:
                            nc.vector.tensor_mul(
                                p_t[:, 0:QB], p_t[:, 0:QB],
                                masks_sb[:, j * QB:(j + 1) * QB],
                            )
                            first = False
                        p_tiles[j].append((p_t, w))
                    return

                def p_slice(j, gq):
                    """stationary [128, 128] slice of p for key tile j, query col gq."""
                    rel = gq - SOFF[j // 4]
                    piece, off = p_tiles[j][rel // 512], rel % 512
                    return piece[0][:, off:off + 128]

                def v_tiles(b):
                    # project the 4 natural [s, dk] v tiles of vx block b:
                    # stationary = vx s-tile (e3m4), moving = Wv chunk (bf16)
                    for t4 in range(4):
                        g = 4 * b + t4
                        ps = psM.tile([128, 128], F32, tag="ps_misc")
                        for c in range(NCH):
                            nc.tensor.matmul(
                                ps,
                                vxb[b][:, c, t4 * 128:(t4 + 1) * 128],
                                wTs["wv"][:, c, :],
                                start=(c == 0),
                                stop=(c == NCH - 1),
                            )
                        nc.vector.tensor_copy(vsb[:, g, 0:DK], ps)
                    nc.vector.memset(vsb[:, 4 * b:4 * b + 4, DK:DK + 1], 1.0)

                o_big = [
                    o_pool.tile([128, 2, DK], BF16, tag=f"ob{s}", name=f"ob{s}", bufs=1)
                    for s in range(NSLOT)
                ]

                def pv(qc):
                    """P@V for query chunk qc (128 rows); divide + store per slot."""
                    slot = qc // 2
                    njq = NJ[slot]
                    gq = qc * 128
                    ps_o = psO.tile([128, VW], F32, tag="out", name=f"ps_o_{qc}")
                    for j in range(njq):
                        nc.tensor.matmul(
                            ps_o, p_slice(j, gq), vsb[:, j, :],
                            start=(j == 0), stop=(j == njq - 1),
                        )
                    rec = o_pool.tile([128, 1], F32, tag="rec")
                    nc.vector.reciprocal(rec, ps_o[:, DK:DK + 1])
                    nc.vector.tensor_scalar_mul(o_big[slot][:, qc % 2, :], ps_o[:, 0:DK], rec)
                    if qc % 2 == 1:
                        (nc.sync if slot % 2 else nc.gpsimd).dma_start(
                            out=out[slot], in_=o_big[slot])

                # ---------- pipeline ----------
                # interleave projections/PV between score groups so the PE
                # stays busy while the serial ACT exp chain catches up
                for blk in range(2):
                    project_block(wTs["wq"], qxb[blk], qT_sb, blk * 512, 0)

                # each K block is followed immediately by the score tiles it
                # unlocks, so score matmuls fill the next block's DMA wait
                for blk in range(4):
                    project_block(wTs["wk"], kxb[blk], kT_sb, blk * 512, 0)
                    for j in range(4 * blk, 4 * blk + 4):
                        scores(j)
                v_tiles(0)
                pv(0)
                pv(1)
                v_tiles(1)
                pv(2)
                pv(3)
                v_tiles(2)
                v_tiles(3)
                for qc in (6, 7, 4, 5):
                    pv(qc)

    nc.compile()
    _CACHE["nc"] = nc
    return nc


def _shift_block(h):
    """[128, 16] f32: mask[p, j, c] = (c >= shift) == (key 128j+p <= query 256b+c)."""
    p = np.arange(128, dtype=np.float32)[:, None]
    j = np.arange(NJT, dtype=np.float32)[None, :]
    qb = np.array([256.0 * BLOCKS[h][int(t) // 4] for t in range(NJT)], np.float32)[None, :]
    return (128.0 * j + p - qb).astype(np.float32)


def kernel(**inputs):
    queries = np.asarray(inputs["queries"], dtype=np.float32)
    keys = np.asarray(inputs["keys"], dtype=np.float32)
    values = np.asarray(inputs["values"], dtype=np.float32)

    nc = _build()
    bf = ml_dtypes.bfloat16
    e3 = ml_dtypes.float8_e3m4
    shifts = [_shift_block(0), _shift_block(1)]
    qrows = [
        np.concatenate([np.arange(QB * b, QB * (b + 1)) for b in BLOCKS[h]])
        for h in range(2)
    ]
    def pack_w(xt):
        # [DM, DK] -> [128, NCH, DK]
        return np.ascontiguousarray(xt.reshape(NCH, 128, -1).transpose(1, 0, 2))

    def pack_x(xt):
        # [DM, cols] -> [nblk, 128, NCH, 512]: one packed block per 512 cols
        nblk = xt.shape[1] // 512
        b = xt.reshape(NCH, 128, nblk, 512).transpose(2, 1, 0, 3)
        return np.ascontiguousarray(b)

    def pack_kv(x):
        return pack_x(x.T)

    wT = {
        nm: pack_w(np.asarray(inputs[nm], dtype=np.float32).T.astype(bf))
        for nm in ("Wq", "Wk", "Wv")
    }
    kxs = [pack_kv(keys[b].astype(e3)) for b in range(B)]
    vxs = [pack_kv(values[b].astype(e3)) for b in range(B)]

    in_maps = []
    for c in range(NCORES):
        b, h = c // 2, c % 2
        in_maps.append(
            {
                "qx": pack_x(queries[b][qrows[h]].T.astype(e3)),
                "kx": kxs[b],
                "vx": vxs[b],
                "wqT": wT["Wq"],
                "wkT": wT["Wk"],
                "wvT": wT["Wv"],
                "shifts": shifts[h],
            }
        )

    res = bass_utils.run_bass_kernel_spmd(
        nc, in_maps, list(range(NCORES)), **_CACHE.get("run_kwargs", {})
    )
    _CACHE["last_result"] = res

    out = np.empty((B, S, DK), dtype=np.float32)
    for c in range(NCORES):
        b, h = c // 2, c % 2
        o4 = np.asarray(res.results[c]["out"], dtype=np.float32)
        out[b][qrows[h]] = o4.transpose(0, 2, 1, 3).reshape(HALF, DK)
    return out


# revision 21
# speedup vs baseline: 1.0866x; 1.0866x over previous
"""Single-head causal attention (B=4, S=2048, D=1024, dk=128) on 8 TRN2 cores.

Sharding: core c -> batch b=c//2, half h=c%2. Query rows are split at
256-row granularity into blocks; core h owns blocks (0,3,4,7) or
(1,2,5,6) of the 8 blocks, ordered by causal reach so the two cores run
one identical program whose per-slot key-tile counts are the positional
max (4,8,12,16) of both cores' needs — 40 key-tiles vs the 48 a
512-block split needs.

Activations are passed pre-transposed to [d_model, s] and pre-cast to
fp8 E3M4 (1 byte/elem, 4 mantissa bits); weights stay bf16. The PE
multiplies bf16 stationary x fp8e3 moving directly (verified exact vs
emulation), so fp8 halves HBM traffic without an upcast pass or extra
matmul error beyond the e3m4 quantization itself (~1.1e-2 rel, vs the
2e-2 gate). Scores are computed transposed ([key, query]) so the P@V
matmul consumes P tiles as the stationary operand and V in natural
[s, dk] layout; a ones-column appended to V makes the same matmul
accumulate the softmax denominators. The causal mask is applied as a
multiplicative bf16 mask on P built on-chip from a per-core [128, 16]
shift table: mask tile j covers query cols [soff(j), soff(j)+256) and
is all-ones / diagonal / all-zero as needed, so one program serves both
halves.
"""

import math

import numpy as np
import ml_dtypes

import concourse.bacc as bacc
import concourse.tile as tile
import concourse.mybir as mybir
from concourse import bass_utils
from concourse.masks import make_identity
from concourse.tile_rust import add_dep_helper

F32 = mybir.dt.float32
BF16 = mybir.dt.bfloat16
E3 = mybir.dt.float8e3

B, S, DM, DK = 4, 2048, 1024, 128
NCORES = 8
HALF = S // 2          # query rows per core
NCH = DM // 128        # d_model chunks
NJT = S // 128         # key tiles (16)
QB = 256               # query block rows
NSLOT = HALF // QB     # 4 query-block slots per core
NJ = (4, 8, 12, 16)    # key tiles computed for slot j//4
SOFF = (0, 256, 512, 768)  # first query col that still needs key tile j
VW = DK + 1            # v tiles carry a ones-column for the denominator
SCALE = 1.0 / math.sqrt(DK)
# per-core query blocks, ordered so both cores' causal needs fit (4,8,12,16)
BLOCKS = ((0, 3, 4, 7), (1, 2, 5, 6))
WARMUP_MMS = 24
FILLER_MMS = 2

_CACHE = {}


def _build():
    if "nc" in _CACHE:
        return _CACHE["nc"]
    nc = bacc.Bacc("TRN2", target_bir_lowering=False, debug=False, num_devices=NCORES)

    # packed host layouts: one [128, NCH, 512] contiguous block per 512
    # activation columns, so each projection block gates only on its own slice
    qx_in = nc.dram_tensor("qx", [2, 128, NCH, 512], E3, kind="ExternalInput").ap()
    kx_in = nc.dram_tensor("kx", [4, 128, NCH, 512], E3, kind="ExternalInput").ap()
    vx_in = nc.dram_tensor("vx", [4, 128, NCH, 512], E3, kind="ExternalInput").ap()
    wqT_in = nc.dram_tensor("wqT", [128, NCH, DK], BF16, kind="ExternalInput").ap()
    wkT_in = nc.dram_tensor("wkT", [128, NCH, DK], BF16, kind="ExternalInput").ap()
    wvT_in = nc.dram_tensor("wvT", [128, NCH, DK], BF16, kind="ExternalInput").ap()
    shifts_in = nc.dram_tensor("shifts", [128, NJT], F32, kind="ExternalInput").ap()
    out = nc.dram_tensor("out", [NSLOT, 128, 2, DK], BF16, kind="ExternalOutput").ap()

    rings = [nc.sync, nc.scalar, nc.gpsimd]

    with tile.TileContext(nc) as tc:
        with tc.tile_pool(name="const", bufs=1) as const:
            ident = const.tile([128, 128], BF16)
            make_identity(nc, ident)

            # ---- loads: per-(chunk, col-block) DMAs (1-2KB contiguous per
            # partition row), round-robined over 3 issue rings in need order.
            wTs = {}
            for nm, w_dram in (("wq", wqT_in), ("wk", wkT_in), ("wv", wvT_in)):
                wTs[nm] = const.tile([128, NCH, DK], BF16, tag=f"wT_{nm}", name=f"wT{nm}")
            shifts = const.tile([128, NJT], F32)
            iota_i = const.tile([128, QB], mybir.dt.int32)
            nc.gpsimd.iota(iota_i, pattern=[[1, QB]], base=0, channel_multiplier=0)
            iota_f = const.tile([128, QB], F32)
            nc.vector.tensor_copy(iota_f, iota_i)

            qxb = [const.tile([128, NCH, 512], E3, tag=f"qx{b}", name=f"qxb{b}") for b in range(2)]
            kxb = [const.tile([128, NCH, 512], E3, tag=f"kx{b}", name=f"kxb{b}") for b in range(4)]
            vxb = [const.tile([128, NCH, 512], E3, tag=f"vx{b}", name=f"vxb{b}") for b in range(4)]
            ri = 0

            def ld(dst, src):
                nonlocal ri
                rings[ri % len(rings)].dma_start(out=dst, in_=src)
                ri += 1

            # scalar (ACT) also runs the exp chain mid-kernel, so its DMA
            # queue is the straggler: give it only the early q slices and the
            # last v slice; stream kx/vx over sync+gpsimd in need order.
            h0, h1 = slice(0, 4), slice(4, 8)

            def ld2(eng, dst, src, cs):
                eng.dma_start(out=dst[:, cs, :], in_=src[:, cs, :])

            nc.scalar.dma_start(out=wTs["wq"], in_=wqT_in)
            nc.gpsimd.dma_start(out=shifts, in_=shifts_in)
            nc.sync.dma_start(out=wTs["wk"], in_=wkT_in)
            for b in range(2):
                ld2(nc.scalar, qxb[b], qx_in[b], h0)
                ld2(nc.gpsimd, qxb[b], qx_in[b], h1)
            for b in range(4):
                ld2(nc.sync, kxb[b], kx_in[b], h0)
                ld2(nc.gpsimd, kxb[b], kx_in[b], h1)
            nc.scalar.dma_start(out=wTs["wv"], in_=wvT_in)
            for b in range(4):
                ld2(nc.sync, vxb[b], vx_in[b], h0)
                ld2(nc.scalar if b == 3 else nc.gpsimd, vxb[b], vx_in[b], h1)

            # ---- causal masks built on-chip: mask[p, j, c] = (c >= shift[p, j])
            masks_sb = const.tile([128, NJT * QB], BF16)
            for j in range(NJT):
                nc.vector.tensor_scalar(
                    masks_sb[:, j * QB:(j + 1) * QB],
                    iota_f,
                    shifts[:, j:j + 1],
                    None,
                    op0=mybir.AluOpType.is_ge,
                )

            # ---- PE warmup + filler: keep the PE p-state ramp going while
            # the first loads land.
            w_warm = const.tile([128, 512], BF16)
            nc.vector.memset(w_warm, 1.0)
            last_filler = None
            with tc.tile_pool(name="psW", bufs=1, space="PSUM") as psW:
                ps_w = psW.tile([128, 512], F32)
                for _ in range(WARMUP_MMS):
                    nc.tensor.matmul(ps_w[:, 0:128], ident, ident, start=True, stop=True)
                for _ in range(FILLER_MMS):
                    last_filler = nc.tensor.matmul(ps_w, ident, w_warm, start=True, stop=True)

            # ---- persistent projected tensors ----
            qT_sb = const.tile([128, HALF], BF16)
            kT_sb = const.tile([128, S], BF16)
            vsb = const.tile([128, NJT, VW], BF16)

            with (
                tc.tile_pool(name="psM", bufs=2, space="PSUM") as psM,
                tc.tile_pool(name="psS", bufs=2, space="PSUM") as psS,
                tc.tile_pool(name="psO", bufs=4, space="PSUM") as psO,
                tc.tile_pool(name="pP", bufs=26) as p_pool,
                tc.tile_pool(name="oo", bufs=4) as o_pool,
            ):

                def project_block(wT, xT, dst, dst0, xoff, w=512):
                    """dst[:, dst0:dst0+w] bf16 = W @ X^T[:, xoff:xoff+w]."""
                    acc = psM.tile([128, 512], F32, tag="ps_misc", name="acc")
                    for c in range(NCH):
                        mm = nc.tensor.matmul(
                            acc[:, 0:w],
                            wT[:, c, :],
                            xT[:, c, xoff:xoff + w],
                            start=(c == 0),
                            stop=(c == NCH - 1),
                        )
                        if c == 0 and last_filler is not None:
                            add_dep_helper(
                                mm.ins, last_filler.ins, sync=False,
                                reason="run filler first",
                            )
                    nc.vector.tensor_copy(dst[:, dst0:dst0 + w], acc[:, 0:w])

                # p pieces: p_tiles[j] = list of (tile, width) covering query
                # cols [soff(j), HALF) in 512-wide pieces
                p_tiles = {j: [] for j in range(NJT)}

                def scores(j):
                    """score tile j over query cols [soff, HALF): [key, query]
                    transposed, exp, causal mask on the first 256 cols."""
                    soff = SOFF[j // 4]
                    kt = kT_sb[:, j * 128:(j + 1) * 128]
                    first = True
                    for q0 in range(soff, HALF, 512):
                        w = min(512, HALF - q0)
                        ps_s = psS.tile([128, 512], F32, tag="score")
                        nc.tensor.matmul(
                            ps_s[:, 0:w], kt, qT_sb[:, q0:q0 + w],
                            start=True, stop=True,
                        )
                        p_t = p_pool.tile([128, 512], BF16, tag="p")
                        nc.scalar.activation(
                            p_t[:, 0:w], ps_s[:, 0:w],
                            mybir.ActivationFunctionType.Exp, scale=SCALE,
                        )
                        if first:
                            nc.vector.tensor_mul(
                                p_t[:, 0:QB], p_t[:, 0:QB],
                                masks_sb[:, j * QB:(j + 1) * QB],
                            )
                            first = False
                        p_tiles[j].append((p_t, w))
                    return

                def p_slice(j, gq):
                    """stationary [128, 128] slice of p for key tile j, query col gq."""
                    rel = gq - SOFF[j // 4]
                    piece, off = p_tiles[j][rel // 512], rel % 512
                    return piece[0][:, off:off + 128]

                def v_natural(h):
                    # project V directly into natural [s, dk] layout:
                    # stationary = vx s-tile (e3m4), moving = Wv chunk (bf16)
                    for t in range(NCH):
                        b = 2 * h + t // 4
                        toff = (t % 4) * 128
                        ps = psM.tile([128, 128], F32, tag="ps_misc")
                        for c in range(NCH):
                            nc.tensor.matmul(
                                ps,
                                vxb[b][:, c, toff:toff + 128],
                                wTs["wv"][:, c, :],
                                start=(c == 0),
                                stop=(c == NCH - 1),
                            )
                        nc.vector.tensor_copy(vsb[:, h * NCH + t, 0:DK], ps)
                    nc.vector.memset(vsb[:, h * NCH:(h + 1) * NCH, DK:DK + 1], 1.0)

                o_big = [
                    o_pool.tile([128, 2, DK], BF16, tag=f"ob{s}", name=f"ob{s}", bufs=1)
                    for s in range(NSLOT)
                ]

                def pv(qc):
                    """P@V for query chunk qc (128 rows); divide + store per slot."""
                    slot = qc // 2
                    njq = NJ[slot]
                    gq = qc * 128
                    ps_o = psO.tile([128, VW], F32, tag="out", name=f"ps_o_{qc}")
                    for j in range(njq):
                        nc.tensor.matmul(
                            ps_o, p_slice(j, gq), vsb[:, j, :],
                            start=(j == 0), stop=(j == njq - 1),
                        )
                    rec = o_pool.tile([128, 1], F32, tag="rec")
                    nc.vector.reciprocal(rec, ps_o[:, DK:DK + 1])
                    nc.vector.tensor_scalar_mul(o_big[slot][:, qc % 2, :], ps_o[:, 0:DK], rec)
                    if qc % 2 == 1:
                        (nc.sync if slot % 2 else nc.gpsimd).dma_start(
                            out=out[slot], in_=o_big[slot])

                # ---------- pipeline ----------
                # interleave projections/PV between score groups so the PE
                # stays busy while the serial ACT exp chain catches up
                for blk in range(2):
                    project_block(wTs["wq"], qxb[blk], qT_sb, blk * 512, 0)

                for blk in range(2):
                    project_block(wTs["wk"], kxb[blk], kT_sb, blk * 512, 0)
                for j in range(8):
                    scores(j)
                for blk in range(2, 4):
                    project_block(wTs["wk"], kxb[blk], kT_sb, blk * 512, 0)
                for j in range(8, NJT):
                    scores(j)
                v_natural(0)
                for qc in range(4):
                    pv(qc)
                v_natural(1)
                for qc in (6, 7, 4, 5):
                    pv(qc)

    nc.compile()
    _CACHE["nc"] = nc
    return nc


def _shift_block(h):
    """[128, 16] f32: mask[p, j, c] = (c >= shift) == (key 128j+p <= query 256b+c)."""
    p = np.arange(128, dtype=np.float32)[:, None]
    j = np.arange(NJT, dtype=np.float32)[None, :]
    qb = np.array([256.0 * BLOCKS[h][int(t) // 4] for t in range(NJT)], np.float32)[None, :]
    return (128.0 * j + p - qb).astype(np.float32)


def kernel(**inputs):
    queries = np.asarray(inputs["queries"], dtype=np.float32)
    keys = np.asarray(inputs["keys"], dtype=np.float32)
    values = np.asarray(inputs["values"], dtype=np.float32)

    nc = _build()
    bf = ml_dtypes.bfloat16
    e3 = ml_dtypes.float8_e3m4
    shifts = [_shift_block(0), _shift_block(1)]
    qrows = [
        np.concatenate([np.arange(QB * b, QB * (b + 1)) for b in BLOCKS[h]])
        for h in range(2)
    ]
    def pack_w(xt):
        # [DM, DK] -> [128, NCH, DK]
        return np.ascontiguousarray(xt.reshape(NCH, 128, -1).transpose(1, 0, 2))

    def pack_x(xt):
        # [DM, cols] -> [nblk, 128, NCH, 512]: one packed block per 512 cols
        nblk = xt.shape[1] // 512
        b = xt.reshape(NCH, 128, nblk, 512).transpose(2, 1, 0, 3)
        return np.ascontiguousarray(b)

    def pack_kv(x):
        return pack_x(x.T)

    wT = {
        nm: pack_w(np.asarray(inputs[nm], dtype=np.float32).T.astype(bf))
        for nm in ("Wq", "Wk", "Wv")
    }
    kxs = [pack_kv(keys[b].astype(e3)) for b in range(B)]
    vxs = [pack_kv(values[b].astype(e3)) for b in range(B)]

    in_maps = []
    for c in range(NCORES):
        b, h = c // 2, c % 2
        in_maps.append(
            {
                "qx": pack_x(queries[b][qrows[h]].T.astype(e3)),
                "kx": kxs[b],
                "vx": vxs[b],
                "wqT": wT["Wq"],
                "wkT": wT["Wk"],
                "wvT": wT["Wv"],
                "shifts": shifts[h],
            }
        )

    res = bass_utils.run_bass_kernel_spmd(
        nc, in_maps, list(range(NCORES)), **_CACHE.get("run_kwargs", {})
    )
    _CACHE["last_result"] = res

    out = np.empty((B, S, DK), dtype=np.float32)
    for c in range(NCORES):
        b, h = c // 2, c % 2
        o4 = np.asarray(res.results[c]["out"], dtype=np.float32)
        out[b][qrows[h]] = o4.transpose(0, 2, 1, 3).reshape(HALF, DK)
    return out
